# revision 1
# baseline (speedup 1.0000x reference)
"""BiCutLoss Trainium2 kernel (nn_BiCutLoss_52312701665760).

Reference computation (per batch row i of output[B, L, 2], labels[B, L]):
  temp = argmax(output, -1)            # 1 iff out1 > out0
  cut  = L if all(temp == 1) else (index of last 0 in temp)
  mask = arange(L) < cut
  r1   = where(labels == 1, -3.6/log2(j+2), 0.065)
  loss = sum(out1 * mask * r1) / B

Kernel formulation (exactly equivalent):
  d[j] = out0[j] - out1[j]                       # temp[j]==0  <=>  d[j] >= 0
  M[j] = max(d[j:], -1)  (reverse cummax; M[L] = -1 pad)
  thr  = 0 if M[0] >= 0 else -BIG                # all-ones row => mask all 1
  mask[j] = (M[j+1] >= thr)
  A_i = sum_j out1*mask          B_i = sum_j out1*mask*labels*(r1pos - 0.065)
  loss_i = 0.065*A_i + B_i

Sharding: pure data parallel — B=4096 rows split as 512 rows x 8 cores; each
core computes per-row partials [128,1] (4 row-tiles of 128 partitions), host
sums and divides by B.

Engines per [128, 4096] row-tile (Pool ucode only implements TT/TS, so
scan / scalar_tensor_tensor / tensor_tensor_reduce all live on DVE):
  DVE   : d[:, :DL] (TT sub), scan, thr, t1m (STT), loss-accum (STT+accum)
  GPSIMD: d[:, DL:] (TT sub), lp = labels_f32 * pre (TT)
  ACT   : rr = lp + 0.065 (activation copy+bias)
  loss_i = sum_j t1m * rr, chained across tiles via TTR initial.
  labels are cast int32->f32 during DMA (SWDGE); HBM traffic unchanged.
"""

import os
from contextlib import ExitStack

import numpy as np

B, L = 4096, 4096
N_CORES = 8
ROWS_PER_CORE = B // N_CORES          # 512
P = 128                               # partitions per tile
TILES = ROWS_PER_CORE // P            # 4
C_CONST = 0.65 * 0.1                  # 0.065
BIG = 1e30

_CACHE = {}


def _build_nc(repeat: int = 1, dl: int = 1408):
    import concourse.mybir as mybir
    import concourse.tile as tile
    from concourse import bacc

    f32 = mybir.dt.float32
    bf16 = mybir.dt.bfloat16
    i32 = mybir.dt.int32
    Op = mybir.AluOpType

    # Bacc (not raw Bass): its compile() runs generate_event_semaphores,
    # which splits multi-sem waits into standalone EventSemaphore
    # instructions (HW allows at most 1 wait per compute instruction).
    nc = bacc.Bacc("TRN2", target_bir_lowering=False, debug=False)

    out_d = nc.dram_tensor("out", [ROWS_PER_CORE, L * 2], f32, kind="ExternalInput")
    lab_d = nc.dram_tensor("lab", [ROWS_PER_CORE, L], i32, kind="ExternalInput")
    pre_d = nc.dram_tensor("pre", [P, L], f32, kind="ExternalInput")
    res_d = nc.dram_tensor("res", [P, 1], f32, kind="ExternalOutput")

    out_t = out_d[:].rearrange("(n p) m -> n p m", p=P)   # [4, 128, 8192]
    lab_t = lab_d[:].rearrange("(n p) m -> n p m", p=P)   # [4, 128, 4096]

    with tile.TileContext(nc) as tc, ExitStack() as ctx:
        io_pool = ctx.enter_context(tc.tile_pool(name="io", bufs=2))
        pre_pool = ctx.enter_context(tc.tile_pool(name="pre", bufs=1))
        d_pool = ctx.enter_context(tc.tile_pool(name="d", bufs=1))
        m_pool = ctx.enter_context(tc.tile_pool(name="m", bufs=1))
        t1m_pool = ctx.enter_context(tc.tile_pool(name="t1m", bufs=2))
        lp_pool = ctx.enter_context(tc.tile_pool(name="lp", bufs=1))
        rr_pool = ctx.enter_context(tc.tile_pool(name="rr", bufs=1))
        acc_pool = ctx.enter_context(tc.tile_pool(name="acc", bufs=1))

        pre_tl = pre_pool.tile([P, L], f32)
        nc.sync.dma_start(pre_tl[:], pre_d[:])

        acc_B = acc_pool.tile([P, TILES], f32)

        for _r in range(repeat):
            for k in range(TILES):
                ot = io_pool.tile([P, L * 2], f32, tag="ot")
                nc.sync.dma_start(ot[:], out_t[k])
                # labels: int32 -> f32 cast during DMA (SWDGE path); Pool
                # rejects int32 operands and HBM read traffic is unchanged.
                lt = io_pool.tile([P, L], f32, tag="lt")
                nc.gpsimd.dma_start(lt[:], lab_t[k])

                x3 = ot[:].rearrange("p (l c) -> p l c", c=2)
                t0 = x3[:, :, 0]
                t1 = x3[:, :, 1]

                # pass 1 (split DVE/Pool): d = t0 - t1
                d = d_pool.tile([P, L], f32)
                nc.vector.tensor_tensor(
                    d[:, 0:dl], t0[:, 0:dl], t1[:, 0:dl], Op.subtract)
                nc.gpsimd.tensor_tensor(
                    d[:, dl:L], t0[:, dl:L], t1[:, dl:L], Op.subtract)

                # pass 2 (DVE): M[j] = max(d[j:], -1), M[L] = -1 (bf16)
                M = m_pool.tile([P, L + 1], bf16)
                nc.vector.memset(M[:, L:L + 1], -1.0)
                nc.vector.tensor_tensor_scan(
                    M[:, 0:L][:, ::-1], d[:, ::-1], d[:, ::-1], -1.0,
                    Op.max, Op.max,
                )

                # tiny (DVE): thr = 0 if M[0] >= 0 else -BIG
                thr = acc_pool.tile([P, 1], f32, tag="thr")
                nc.vector.tensor_scalar(
                    thr[:], M[:, 0:1], 0.0, BIG, Op.is_ge, Op.mult
                )
                nc.vector.tensor_scalar_add(thr[:], thr[:], -BIG)

                # pass 3 (DVE): t1m = (M[j+1] >= thr) * t1
                t1m = t1m_pool.tile([P, L], f32)
                nc.vector.scalar_tensor_tensor(
                    t1m[:], M[:, 1:L + 1], thr[:], t1,
                    Op.is_ge, Op.mult,
                )

                # pass 4 (GPSIMD): lp = lab_f32 * pre
                lp = lp_pool.tile([P, L], f32)
                nc.gpsimd.tensor_tensor(lp[:], lt[:], pre_tl[:], Op.mult)

                # pass 5 (ACT): rr = lp + 0.065
                rr = rr_pool.tile([P, L], f32)
                nc.scalar.activation(
                    rr[:], lp[:],
                    mybir.ActivationFunctionType.Copy,
                    bias=C_CONST, scale=1.0,
                )

                # pass 6 (DVE): loss_k = sum(t1m * rr)
                # (tensor_tensor_reduce crashes TRN2 HW; STT+accum_out works)
                # Main output written in-place into t1m (1:1 streaming) so d
                # is not written here — otherwise Pool's next d-half would
                # serialize against this op (WAR), ping-ponging the engines.
                nc.vector.scalar_tensor_tensor(
                    t1m[:], t1m[:], 1.0, rr[:], Op.mult, Op.mult,
                    accum_out=acc_B[:, k:k + 1],
                )

            # tail: loss_i = sum_k loss_k
            loss_t = acc_pool.tile([P, 1], f32, tag="loss")
            nc.vector.reduce_sum(loss_t[:], acc_B[:], axis=mybir.AxisListType.X)

        nc.sync.dma_start(res_d[:], loss_t[:])

    nc.compile()
    return nc


def _pre_tile() -> np.ndarray:
    j = np.arange(L, dtype=np.float64)
    pre2 = (-3.6 / np.log2(j + 2.0) - C_CONST).astype(np.float32)
    return np.ascontiguousarray(np.tile(pre2[None, :], (P, 1)))


def _get_nc(repeat: int = 1):
    key = repeat
    if key not in _CACHE:
        _CACHE[key] = _build_nc(repeat=repeat)
    return _CACHE[key]


def make_in_maps(output: np.ndarray, labels: np.ndarray):
    pre = _pre_tile()
    in_maps = []
    for c in range(N_CORES):
        sl = slice(c * ROWS_PER_CORE, (c + 1) * ROWS_PER_CORE)
        in_maps.append({
            "out": np.ascontiguousarray(output[sl]).reshape(ROWS_PER_CORE, L * 2),
            "lab": np.ascontiguousarray(labels[sl]),
            "pre": pre,
        })
    return in_maps


def kernel(output: np.ndarray, labels: np.ndarray) -> np.ndarray:
    from concourse.bass_utils import run_bass_kernel_spmd

    nc = _get_nc(repeat=1)
    in_maps = make_in_maps(output, labels)
    r = run_bass_kernel_spmd(nc, in_maps, core_ids=list(range(N_CORES)))
    total = 0.0
    for res in r.results:
        total += float(res["res"].astype(np.float64).sum())
    return np.float32(total / B)


if __name__ == "__main__":
    # quick standalone run (full inputs, random)
    rng = np.random.default_rng(0)
    out = rng.standard_normal((B, L, 2)).astype(np.float32)
    lab = rng.integers(0, 2, size=(B, L)).astype(np.int32)
    print("loss:", kernel(out, lab))



# revision 12
# speedup vs baseline: 1.3955x; 1.3955x over previous
"""BiCutLoss Trainium2 kernel (nn_BiCutLoss_52312701665760), v4.

Reference computation (per batch row i of output[B, L, 2], labels[B, L]):
  temp = argmax(output, -1)            # 1 iff out1 > out0
  cut  = L if all(temp == 1) else (index of last 0 in temp)
  mask = arange(L) < cut
  r1   = where(labels == 1, -3.6/log2(j+2), 0.065)
  loss = sum(output[...,1] * mask * r1) / B

Kernel formulation (exactly equivalent):
  d[j]   = out0[j] - out1[j]                    # temp[j]==0  <=>  d[j] >= 0
  M[j]   = max(d[j:], -1)  (reverse cummax; M[L] = -1 pad)
  mask0  = (M[j+1] >= 0)                        # mask ignoring all-ones rows
  v[j]   = (lab*pre2 + C) * out1[j]             # = r1 * out1,  C = 0.065,
                                                #   pre2 = -3.6/log2(j+2) - C
  S1 = sum(mask0 * v); S2 = sum(v); a = [M[0] < 0]   (per row)
  loss_i = S1 + a*(S2 - S1)        # all-negative row => mask all ones => S2

Sharding: pure data parallel - B=4096 rows split as 512 rows x 8 cores; each
core computes per-row partials, host combines (trivial) and divides by B.

Per-core schedule: 4 row-tiles of [128 partitions x 4096 cols]; each tile is
loaded and processed in 4 column chunks RIGHT-TO-LEFT so the reverse-max
scan chains across chunks and compute starts as soon as the first chunk
lands (the kernel is DMA-bound; DVE is the tail constraint otherwise).
  DMA   : pre2 first (gates zl), then per tile: labels (i32->bf16 SWDGE
          cast), out columns as 4 reversed 1 MB HWDGE chunks.
  Pool  : d = t0 - t1 per chunk (strided f32 views -> dense bf16).
  DVE   : zl = lab*pre2 (TT bf16 dense, 2x); per chunk: v = (zl+C)*t1
          (STT, accum_out -> S2 partial), chained reverse-max scan -> M,
          mask = (M[1:] >= 0)*v in-place (STT, accum_out -> S1 partial);
          then a = (M[0] < 0) per tile. STT/scan have no fast uops (always
          1x), so strided/mixed operands there cost nothing extra.
Host: loss_tile = S1 + a*(S2 - S1) summed over partials, / B.
"""

import numpy as np
from contextlib import ExitStack

B, L = 4096, 4096
N_CORES = 8
ROWS_PER_CORE = B // N_CORES          # 512
P = 128                               # partitions per tile
TILES = ROWS_PER_CORE // P            # 4
NCH = 4                               # column chunks per tile
CW = L // NCH                         # 1024 cols per chunk
C_CONST = 0.65 * 0.1                  # 0.065
NSUM = TILES * NCH                    # 16 partial-sum columns per S
RES_COLS = 2 * NSUM + TILES           # S1 | S2 | a  -> 36

_CACHE = {}


def _build_nc(repeat: int = 1):
    import concourse.mybir as mybir
    import concourse.tile as tile
    from concourse import bacc

    f32 = mybir.dt.float32
    bf16 = mybir.dt.bfloat16
    i32 = mybir.dt.int32
    Op = mybir.AluOpType

    nc = bacc.Bacc("TRN2", target_bir_lowering=False, debug=False)

    out_d = nc.dram_tensor("out", [ROWS_PER_CORE, L * 2], f32, kind="ExternalInput")
    lab_d = nc.dram_tensor("lab", [ROWS_PER_CORE, L], i32, kind="ExternalInput")
    pre_d = nc.dram_tensor("pre", [P, L], bf16, kind="ExternalInput")
    res_d = nc.dram_tensor("res", [P, RES_COLS], f32, kind="ExternalOutput")

    out_t = out_d[:].rearrange("(n p) m -> n p m", p=P)   # [4, 128, 8192]
    lab_t = lab_d[:].rearrange("(n p) m -> n p m", p=P)   # [4, 128, 4096]

    with tile.TileContext(nc) as tc, ExitStack() as ctx:
        ot_pool = ctx.enter_context(tc.tile_pool(name="ot", bufs=3))
        lab_pool = ctx.enter_context(tc.tile_pool(name="lab", bufs=4))
        pre_pool = ctx.enter_context(tc.tile_pool(name="pre", bufs=1))
        d_pool = ctx.enter_context(tc.tile_pool(name="d", bufs=2))
        m_pool = ctx.enter_context(tc.tile_pool(name="m", bufs=2))
        v_pool = ctx.enter_context(tc.tile_pool(name="v", bufs=2))
        acc_pool = ctx.enter_context(tc.tile_pool(name="acc", bufs=1))

        pre_tl = pre_pool.tile([P, L], bf16)

        for _r in range(repeat):
            acc = acc_pool.tile([P, RES_COLS], f32, tag="acc")

            def issue_lab(k, first=False):
                if first:
                    # pre2 gates zl0 -> issue before everything else
                    # (host supplies bf16, so this rides the HWDGE queue)
                    nc.sync.dma_start(pre_tl[:], pre_d[:])
                lt = lab_pool.tile([P, L], bf16, tag="lt")
                nc.gpsimd.dma_start(lt[:], lab_t[k])     # i32 -> bf16 cast
                return lt

            def issue_ot(k):
                ots = [None] * NCH
                for c in range(NCH - 1, -1, -1):
                    o = ot_pool.tile([P, 2 * CW], f32, tag=f"ot_{c}")
                    nc.sync.dma_start(
                        o[:], out_t[k][:, 2 * CW * c:2 * CW * (c + 1)])
                    ots[c] = o
                return ots

            def compute(k, lt, ots):
                M = m_pool.tile([P, L + 1], bf16, tag="M")
                nc.vector.memset(M[:, L:L + 1], -1.0)
                # zl = lab * pre2 (bf16 dense, 2x mode), in place over lab
                nc.vector.tensor_tensor(lt[:], lt[:], pre_tl[:], Op.mult)
                for c in range(NCH - 1, -1, -1):
                    a0, b0 = CW * c, CW * (c + 1)
                    x3 = ots[c][:].rearrange("p (l c) -> p l c", c=2)
                    t0, t1 = x3[:, :, 0], x3[:, :, 1]
                    # Pool: d = t0 - t1
                    d = d_pool.tile([P, CW], bf16, tag=f"d_{c}")
                    nc.gpsimd.tensor_tensor(d[:], t0, t1, Op.subtract)
                    # DVE: v = (zl + C) * t1, accum -> S2 partial
                    v = v_pool.tile([P, CW], bf16, tag=f"v_{c}")
                    col = NSUM + k * NCH + c
                    nc.vector.scalar_tensor_tensor(
                        v[:], lt[:, a0:b0], C_CONST, t1, Op.add, Op.mult,
                        accum_out=acc[:, col:col + 1])
                    # DVE: chained reverse-max scan
                    nc.vector.tensor_tensor_scan(
                        M[:, a0:b0][:, ::-1], d[:, ::-1], d[:, ::-1],
                        M[:, b0:b0 + 1], Op.max, Op.max)
                    # DVE: mask*v in place, accum -> S1 partial
                    col = k * NCH + c
                    nc.vector.scalar_tensor_tensor(
                        v[:], M[:, a0 + 1:b0 + 1], 0.0, v[:],
                        Op.is_ge, Op.mult, accum_out=acc[:, col:col + 1])
                # a_k = (M[0] < 0)
                nc.vector.tensor_scalar(
                    acc[:, 2 * NSUM + k:2 * NSUM + k + 1], M[:, 0:1],
                    0.0, 1.0, Op.is_lt, Op.mult)

            # all labels upfront (bufs=4; cheap, and zl_k gates each tile's
            # DVE block), ot chunks two tiles deep
            lts = [issue_lab(k, first=(_r == 0 and k == 0))
                   for k in range(TILES)]
            ots = [issue_ot(0), issue_ot(1)]
            for k in range(TILES):
                compute(k, lts[k], ots[k])
                if k + 2 < TILES:
                    ots.append(issue_ot(k + 2))

        nc.sync.dma_start(res_d[:], acc[:])

    nc.compile()
    return nc


def _pre_tile() -> np.ndarray:
    import ml_dtypes
    j = np.arange(L, dtype=np.float64)
    pre2 = (-3.6 / np.log2(j + 2.0) - C_CONST).astype(ml_dtypes.bfloat16)
    return np.ascontiguousarray(np.tile(pre2[None, :], (P, 1)))


def _get_nc(repeat: int = 1):
    key = repeat
    if key not in _CACHE:
        _CACHE[key] = _build_nc(repeat=repeat)
    return _CACHE[key]


def make_in_maps(output: np.ndarray, labels: np.ndarray):
    pre = _pre_tile()
    in_maps = []
    for c in range(N_CORES):
        sl = slice(c * ROWS_PER_CORE, (c + 1) * ROWS_PER_CORE)
        in_maps.append({
            "out": np.ascontiguousarray(output[sl]).reshape(ROWS_PER_CORE, L * 2),
            "lab": np.ascontiguousarray(labels[sl]),
            "pre": pre,
        })
    return in_maps


def combine_res(res: np.ndarray) -> float:
    """res: [P, RES_COLS] -> this core's total loss sum (fp64 on host).

    Layout: S1 partials [0:16) (tile k chunk c at k*4+c), S2 [16:32),
    a flags [32:36) per tile.
    """
    r = res.astype(np.float64)
    s1 = r[:, 0:NSUM].reshape(P, TILES, NCH).sum(axis=2)
    s2 = r[:, NSUM:2 * NSUM].reshape(P, TILES, NCH).sum(axis=2)
    a = r[:, 2 * NSUM:2 * NSUM + TILES]
    return float((s1 + a * (s2 - s1)).sum())


def kernel(output: np.ndarray, labels: np.ndarray) -> np.ndarray:
    from concourse.bass_utils import run_bass_kernel_spmd

    nc = _get_nc(repeat=1)
    in_maps = make_in_maps(output, labels)
    r = run_bass_kernel_spmd(nc, in_maps, core_ids=list(range(N_CORES)))
    total = 0.0
    for res in r.results:
        total += combine_res(res["res"])
    return np.float32(total / B)


if __name__ == "__main__":
    # quick standalone run (full inputs, random)
    rng = np.random.default_rng(0)
    out = rng.standard_normal((B, L, 2)).astype(np.float32)
    lab = rng.integers(0, 2, size=(B, L)).astype(np.int32)
    print("loss:", kernel(out, lab))


# revision 16
# speedup vs baseline: 1.7741x; 1.2713x over previous
"""BiCutLoss Trainium2 kernel (nn_BiCutLoss_52312701665760), v5.

Reference computation (per batch row i of output[B, L, 2], labels[B, L]):
  temp = argmax(output, -1)            # 1 iff out1 > out0
  cut  = L if all(temp == 1) else (index of last 0 in temp)
  mask = arange(L) < cut
  r1   = where(labels == 1, -3.6/log2(j+2), 0.065)
  loss = sum(output[...,1] * mask * r1) / B

Kernel formulation (exactly equivalent):
  d[j]   = out0[j] - out1[j]                    # temp[j]==0  <=>  d[j] >= 0
  M[j]   = max(d[j:], -1)  (reverse cummax; M[L] = -1 pad)
  mb     = (M[j+1] >= 0)                        # mask ignoring all-ones rows
  v[j]   = (lab*pre2 + C) * out1[j]             # = r1 * out1,  C = 0.065,
                                                #   pre2 = -3.6/log2(j+2) - C
  S1 = sum(mb * v); S2 = sum(v); a = [M[0] < 0]      (per row)
  loss_i = S1 + a*(S2 - S1)        # all-negative row => mask all ones => S2

Sharding: pure data parallel - B=4096 rows split as 512 rows x 8 cores; each
core computes per-row partials, host combines (trivial) and divides by B.

The kernel is COMPUTE-bound on DVE (measured DMA floor for the 24 MB/core is
only ~42-45 us), so work is spread across all four engines:
  DMA   : per tile: labels (i32->bf16 SWDGE cast), out as 4 reversed 1 MB
          HWDGE column chunks (reverse so the scan can chain right-to-left).
  Pool  : d = t0 - t1 per chunk (strided f32 views -> dense bf16).
  ACT   : t1d = dense bf16 copy of strided out1 (enables 2x DVE TTs),
          zlc = zl + C (bias slot), S2 = Copy(v) accum_out,
          S1 = Copy(mbv) accum_out.
  DVE   : zl = lab*pre2 (TT bf16 2x), v = zlc*t1d (TT 2x),
          chained reverse-max scan -> M (1x, no fast uop exists),
          mb = (M[1:] >= 0) (TS 4x), mbv = mb*v (TT 2x, in place over mb),
          a = (M[0] < 0) tiny TS.
Engine busy per iteration (model): DVE ~52, DMA ~42-47, ACT ~40, Pool ~38.
The last tile runs v/mb/mbv per chunk so only ~5 us trails the final byte.
"""

import numpy as np
from contextlib import ExitStack

B, L = 4096, 4096
N_CORES = 8
ROWS_PER_CORE = B // N_CORES          # 512
P = 128                               # partitions per tile
TILES = ROWS_PER_CORE // P            # 4
NCH = 4                               # column chunks per tile
CW = L // NCH                         # 1024 cols per chunk
C_CONST = 0.65 * 0.1                  # 0.065
NSUM = TILES - 1 + NCH                # 7 partial-sum columns per S
RES_COLS = 2 * NSUM + TILES           # S1 | S2 | a  -> 18

_CACHE = {}


def _build_nc(repeat: int = 1):
    import concourse.mybir as mybir
    import concourse.tile as tile
    from concourse import bacc

    f32 = mybir.dt.float32
    bf16 = mybir.dt.bfloat16
    i32 = mybir.dt.int32
    Op = mybir.AluOpType
    Act = mybir.ActivationFunctionType

    nc = bacc.Bacc("TRN2", target_bir_lowering=False, debug=False)

    out_d = nc.dram_tensor("out", [ROWS_PER_CORE, L * 2], f32, kind="ExternalInput")
    lab_d = nc.dram_tensor("lab", [ROWS_PER_CORE, L], i32, kind="ExternalInput")
    pre_d = nc.dram_tensor("pre", [P, L], bf16, kind="ExternalInput")
    res_d = nc.dram_tensor("res", [P, RES_COLS], f32, kind="ExternalOutput")

    out_t = out_d[:].rearrange("(n p) m -> n p m", p=P)   # [4, 128, 8192]
    lab_t = lab_d[:].rearrange("(n p) m -> n p m", p=P)   # [4, 128, 4096]

    with tile.TileContext(nc) as tc, ExitStack() as ctx:
        ot_pool = ctx.enter_context(tc.tile_pool(name="ot", bufs=2))
        lab_pool = ctx.enter_context(tc.tile_pool(name="lab", bufs=4))
        pre_pool = ctx.enter_context(tc.tile_pool(name="pre", bufs=1))
        d_pool = ctx.enter_context(tc.tile_pool(name="d", bufs=2))
        m_pool = ctx.enter_context(tc.tile_pool(name="m", bufs=2))
        t1_pool = ctx.enter_context(tc.tile_pool(name="t1", bufs=2))
        v_pool = ctx.enter_context(tc.tile_pool(name="v", bufs=2))
        mb_pool = ctx.enter_context(tc.tile_pool(name="mb", bufs=2))
        acc_pool = ctx.enter_context(tc.tile_pool(name="acc", bufs=1))

        pre_tl = pre_pool.tile([P, L], bf16)

        for _r in range(repeat):
            acc = acc_pool.tile([P, RES_COLS], f32, tag="acc")

            def issue_lab(k, first=False):
                if first:
                    # pre2 gates zl0 -> issue before everything else
                    # (host supplies bf16, so this rides the HWDGE queue)
                    nc.sync.dma_start(pre_tl[:], pre_d[:])
                lt = lab_pool.tile([P, L], bf16, tag="lt")
                nc.gpsimd.dma_start(lt[:], lab_t[k])     # i32 -> bf16 cast
                return lt

            def issue_ot(k):
                ots = [None] * NCH
                for c in range(NCH - 1, -1, -1):
                    o = ot_pool.tile([P, 2 * CW], f32, tag=f"ot_{c}")
                    nc.sync.dma_start(
                        o[:], out_t[k][:, 2 * CW * c:2 * CW * (c + 1)])
                    ots[c] = o
                return ots

            def compute(k, lt, ots, chunked):
                M = m_pool.tile([P, L + 1], bf16, tag="M")
                nc.vector.memset(M[:, L:L + 1], -1.0)
                # DVE: zl = lab * pre2 (bf16 dense, 2x), in place over lab
                nc.vector.tensor_tensor(lt[:], lt[:], pre_tl[:], Op.mult)
                # ACT: zlc = zl + C, in place
                nc.scalar.activation(lt[:], lt[:], Act.Copy, bias=C_CONST)
                t1d = t1_pool.tile([P, L], bf16, tag="t1d")
                dt = d_pool.tile([P, L], bf16, tag="d")
                for c in range(NCH - 1, -1, -1):
                    a0, b0 = CW * c, CW * (c + 1)
                    x3 = ots[c][:].rearrange("p (l c) -> p l c", c=2)
                    t0, t1 = x3[:, :, 0], x3[:, :, 1]
                    # ACT: dense bf16 copy of out1
                    nc.scalar.activation(t1d[:, a0:b0], t1, Act.Copy)
                    # Pool: d = t0 - t1
                    nc.gpsimd.tensor_tensor(dt[:, a0:b0], t0, t1, Op.subtract)

                vt = v_pool.tile([P, L], bf16, tag="v")
                mbt = mb_pool.tile([P, L], bf16, tag="mb")

                def vmask(a0, b0, s1col, s2col):
                    v = vt[:, a0:b0]
                    mb = mbt[:, a0:b0]
                    # DVE: v = zlc * t1d (TT 2x)
                    nc.vector.tensor_tensor(
                        v, lt[:, a0:b0], t1d[:, a0:b0], Op.mult)
                    # ACT: S2 partial
                    nc.scalar.activation(
                        v, v, Act.Copy,
                        accum_out=acc[:, NSUM + s2col:NSUM + s2col + 1])
                    # DVE: mb = (M[j+1] >= 0) (TS 4x)
                    nc.vector.tensor_scalar(
                        mb, M[:, a0 + 1:b0 + 1], 0.0, 1.0,
                        Op.is_ge, Op.mult)
                    # DVE: mbv = mb * v (TT 2x, in place over mb)
                    nc.vector.tensor_tensor(mb, mb, v, Op.mult)
                    # ACT: S1 partial
                    nc.scalar.activation(
                        mb, mb, Act.Copy,
                        accum_out=acc[:, s1col:s1col + 1])

                if chunked:
                    for c in range(NCH - 1, -1, -1):
                        a0, b0 = CW * c, CW * (c + 1)
                        # DVE: chained reverse-max scan
                        nc.vector.tensor_tensor_scan(
                            M[:, a0:b0][:, ::-1],
                            dt[:, a0:b0][:, ::-1], dt[:, a0:b0][:, ::-1],
                            M[:, b0:b0 + 1], Op.max, Op.max)
                        vmask(a0, b0, 3 + c, 3 + c)
                else:
                    nc.vector.tensor_tensor_scan(
                        M[:, 0:L][:, ::-1], dt[:, ::-1], dt[:, ::-1],
                        -1.0, Op.max, Op.max)
                    vmask(0, L, k, k)
                # a_k = (M[0] < 0)
                nc.vector.tensor_scalar(
                    acc[:, 2 * NSUM + k:2 * NSUM + k + 1], M[:, 0:1],
                    0.0, 1.0, Op.is_lt, Op.mult)

            # all labels upfront (bufs=4; zl_k gates each tile's DVE block),
            # ot chunks two tiles deep
            lts = [issue_lab(k, first=(_r == 0 and k == 0))
                   for k in range(TILES)]
            ots = [issue_ot(0), issue_ot(1)]
            for k in range(TILES):
                compute(k, lts[k], ots[k], chunked=(k == TILES - 1))
                if k + 2 < TILES:
                    ots.append(issue_ot(k + 2))

        nc.sync.dma_start(res_d[:], acc[:])

    nc.compile()
    return nc


def _pre_tile() -> np.ndarray:
    import ml_dtypes
    j = np.arange(L, dtype=np.float64)
    pre2 = (-3.6 / np.log2(j + 2.0) - C_CONST).astype(ml_dtypes.bfloat16)
    return np.ascontiguousarray(np.tile(pre2[None, :], (P, 1)))


def _get_nc(repeat: int = 1):
    key = repeat
    if key not in _CACHE:
        _CACHE[key] = _build_nc(repeat=repeat)
    return _CACHE[key]


def make_in_maps(output: np.ndarray, labels: np.ndarray):
    pre = _pre_tile()
    in_maps = []
    for c in range(N_CORES):
        sl = slice(c * ROWS_PER_CORE, (c + 1) * ROWS_PER_CORE)
        in_maps.append({
            "out": np.ascontiguousarray(output[sl]).reshape(ROWS_PER_CORE, L * 2),
            "lab": np.ascontiguousarray(labels[sl]),
            "pre": pre,
        })
    return in_maps


def combine_res(res: np.ndarray) -> float:
    """res: [P, RES_COLS] -> this core's total loss sum (fp64 on host).

    Layout: S1 partials [0:7) = tiles 0-2 then tile-3 chunks c0..c3;
    S2 partials [7:14) same order; a flags [14:18) per tile.
    """
    r = res.astype(np.float64)
    s1 = np.stack([r[:, 0], r[:, 1], r[:, 2], r[:, 3:NSUM].sum(axis=1)], axis=1)
    s2 = np.stack([r[:, NSUM], r[:, NSUM + 1], r[:, NSUM + 2],
                   r[:, NSUM + 3:2 * NSUM].sum(axis=1)], axis=1)
    a = r[:, 2 * NSUM:2 * NSUM + TILES]
    return float((s1 + a * (s2 - s1)).sum())


def kernel(output: np.ndarray, labels: np.ndarray) -> np.ndarray:
    from concourse.bass_utils import run_bass_kernel_spmd

    nc = _get_nc(repeat=1)
    in_maps = make_in_maps(output, labels)
    r = run_bass_kernel_spmd(nc, in_maps, core_ids=list(range(N_CORES)))
    total = 0.0
    for res in r.results:
        total += combine_res(res["res"])
    return np.float32(total / B)


if __name__ == "__main__":
    # quick standalone run (full inputs, random)
    rng = np.random.default_rng(0)
    out = rng.standard_normal((B, L, 2)).astype(np.float32)
    lab = rng.integers(0, 2, size=(B, L)).astype(np.int32)
    print("loss:", kernel(out, lab))


# revision 20
# speedup vs baseline: 1.9553x; 1.1021x over previous
"""BiCutLoss Trainium2 kernel (nn_BiCutLoss_52312701665760), v5.

Reference computation (per batch row i of output[B, L, 2], labels[B, L]):
  temp = argmax(output, -1)            # 1 iff out1 > out0
  cut  = L if all(temp == 1) else (index of last 0 in temp)
  mask = arange(L) < cut
  r1   = where(labels == 1, -3.6/log2(j+2), 0.065)
  loss = sum(output[...,1] * mask * r1) / B

Kernel formulation (exactly equivalent):
  d[j]   = out0[j] - out1[j]                    # temp[j]==0  <=>  d[j] >= 0
  M[j]   = max(d[j:], -1)  (reverse cummax; M[L] = -1 pad)
  mb     = (M[j+1] >= 0)                        # mask ignoring all-ones rows
  v[j]   = (lab*pre2 + C) * out1[j]             # = r1 * out1,  C = 0.065,
                                                #   pre2 = -3.6/log2(j+2) - C
  S1 = sum(mb * v); S2 = sum(v); a = [M[0] < 0]      (per row)
  loss_i = S1 + a*(S2 - S1)        # all-negative row => mask all ones => S2

Sharding: pure data parallel - B=4096 rows split as 512 rows x 8 cores; each
core computes per-row partials, host combines (trivial) and divides by B.

The kernel is COMPUTE-bound on DVE (measured DMA floor for the 24 MB/core is
only ~42-45 us), so work is spread across all four engines:
  DMA   : per tile: labels (i32->bf16 SWDGE cast), out as 4 reversed 1 MB
          HWDGE column chunks (reverse so the scan can chain right-to-left).
  Pool  : d = t0 - t1 per chunk (strided f32 views -> dense bf16).
  ACT   : t1d = dense bf16 copy of strided out1 (enables 2x DVE TTs),
          zlc = zl + C (bias slot), S2 = Copy(v) accum_out,
          S1 = Copy(mbv) accum_out.
  DVE   : zl = lab*pre2 (TT bf16 2x), v = zlc*t1d (TT 2x),
          chained reverse-max scan -> M (1x, no fast uop exists),
          mb = (M[1:] >= 0) (TS 4x), mbv = mb*v (TT 2x, in place over mb),
          a = (M[0] < 0) tiny TS.
Engine busy per iteration (model): DVE ~52, DMA ~42-47, ACT ~40, Pool ~38.
The last tile runs v/mb/mbv per chunk so only ~5 us trails the final byte.
"""

import numpy as np
from contextlib import ExitStack

B, L = 4096, 4096
N_CORES = 8
ROWS_PER_CORE = B // N_CORES          # 512
P = 128                               # partitions per tile
TILES = ROWS_PER_CORE // P            # 4
NCH = 4                               # column chunks per tile
CW = L // NCH                         # 1024 cols per chunk
C_CONST = 0.65 * 0.1                  # 0.065
NSUM = TILES - 1 + NCH                # 7 partial-sum columns per S
RES_COLS = 2 * NSUM + TILES           # S1 | S2 | a  -> 18

_CACHE = {}


def _build_nc(repeat: int = 1):
    import concourse.mybir as mybir
    import concourse.tile as tile
    from concourse import bacc

    f32 = mybir.dt.float32
    bf16 = mybir.dt.bfloat16
    i32 = mybir.dt.int32
    Op = mybir.AluOpType
    Act = mybir.ActivationFunctionType

    nc = bacc.Bacc("TRN2", target_bir_lowering=False, debug=False)

    out_d = nc.dram_tensor("out", [ROWS_PER_CORE, L * 2], f32, kind="ExternalInput")
    lab_d = nc.dram_tensor("lab", [ROWS_PER_CORE, L], i32, kind="ExternalInput")
    pre_d = nc.dram_tensor("pre", [P, L], bf16, kind="ExternalInput")
    res_d = nc.dram_tensor("res", [P, RES_COLS], f32, kind="ExternalOutput")

    out_t = out_d[:].rearrange("(n p) m -> n p m", p=P)   # [4, 128, 8192]
    lab_t = lab_d[:].rearrange("(n p) m -> n p m", p=P)   # [4, 128, 4096]

    with tile.TileContext(nc) as tc, ExitStack() as ctx:
        ot_pool = ctx.enter_context(tc.tile_pool(name="ot", bufs=2))
        lab_pool = ctx.enter_context(tc.tile_pool(name="lab", bufs=4))
        pre_pool = ctx.enter_context(tc.tile_pool(name="pre", bufs=1))
        d_pool = ctx.enter_context(tc.tile_pool(name="d", bufs=2))
        m_pool = ctx.enter_context(tc.tile_pool(name="m", bufs=2))
        t1_pool = ctx.enter_context(tc.tile_pool(name="t1", bufs=2))
        v_pool = ctx.enter_context(tc.tile_pool(name="v", bufs=2))
        mb_pool = ctx.enter_context(tc.tile_pool(name="mb", bufs=2))
        acc_pool = ctx.enter_context(tc.tile_pool(name="acc", bufs=1))

        pre_tl = pre_pool.tile([P, L], bf16)

        for _r in range(repeat):
            acc = acc_pool.tile([P, RES_COLS], f32, tag="acc")

            def issue_lab(k, first=False):
                if first:
                    # pre2 gates zl0 -> issue before everything else
                    # (host supplies bf16, so this rides the HWDGE queue)
                    nc.sync.dma_start(pre_tl[:], pre_d[:])
                lt = lab_pool.tile([P, L], bf16, tag="lt")
                nc.gpsimd.dma_start(lt[:], lab_t[k])     # i32 -> bf16 cast
                return lt

            def issue_ot(k):
                ots = [None] * NCH
                for c in range(NCH - 1, -1, -1):
                    o = ot_pool.tile([P, 2 * CW], f32, tag=f"ot_{c}")
                    nc.sync.dma_start(
                        o[:], out_t[k][:, 2 * CW * c:2 * CW * (c + 1)])
                    ots[c] = o
                return ots

            def compute(k, lt, ots, chunked):
                M = m_pool.tile([P, L + 1], bf16, tag="M")
                nc.vector.memset(M[:, L:L + 1], -1.0)
                # DVE: zl = lab * pre2 (bf16 dense, 2x), in place over lab
                nc.vector.tensor_tensor(lt[:], lt[:], pre_tl[:], Op.mult)
                # ACT: zlc = zl + C, in place
                nc.scalar.activation(lt[:], lt[:], Act.Copy, bias=C_CONST)
                t1d = t1_pool.tile([P, L], bf16, tag="t1d")
                dt = d_pool.tile([P, L], bf16, tag="d")
                for c in range(NCH - 1, -1, -1):
                    a0, b0 = CW * c, CW * (c + 1)
                    x3 = ots[c][:].rearrange("p (l c) -> p l c", c=2)
                    t0, t1 = x3[:, :, 0], x3[:, :, 1]
                    # ACT: dense bf16 copy of out1
                    nc.scalar.activation(t1d[:, a0:b0], t1, Act.Copy)
                    # Pool: d = t0 - t1
                    nc.gpsimd.tensor_tensor(dt[:, a0:b0], t0, t1, Op.subtract)

                vt = v_pool.tile([P, L], bf16, tag="v")
                mbt = mb_pool.tile([P, L], bf16, tag="mb")

                def vmask(a0, b0, s1col, s2col):
                    v = vt[:, a0:b0]
                    mb = mbt[:, a0:b0]
                    # DVE: v = zlc * t1d (TT 2x)
                    nc.vector.tensor_tensor(
                        v, lt[:, a0:b0], t1d[:, a0:b0], Op.mult)
                    # ACT: S2 partial
                    nc.scalar.activation(
                        v, v, Act.Copy,
                        accum_out=acc[:, NSUM + s2col:NSUM + s2col + 1])
                    # DVE: mb = (M[j+1] >= 0) (TS 4x)
                    nc.vector.tensor_scalar(
                        mb, M[:, a0 + 1:b0 + 1], 0.0, 1.0,
                        Op.is_ge, Op.mult)
                    # DVE: mbv = mb * v (TT 2x, in place over mb)
                    nc.vector.tensor_tensor(mb, mb, v, Op.mult)
                    # ACT: S1 partial
                    nc.scalar.activation(
                        mb, mb, Act.Copy,
                        accum_out=acc[:, s1col:s1col + 1])

                if chunked:
                    for c in range(NCH - 1, -1, -1):
                        a0, b0 = CW * c, CW * (c + 1)
                        # DVE: chained reverse-max scan
                        nc.vector.tensor_tensor_scan(
                            M[:, a0:b0][:, ::-1],
                            dt[:, a0:b0][:, ::-1], dt[:, a0:b0][:, ::-1],
                            M[:, b0:b0 + 1], Op.max, Op.max)
                        vmask(a0, b0, 3 + c, 3 + c)
                else:
                    nc.vector.tensor_tensor_scan(
                        M[:, 0:L][:, ::-1], dt[:, ::-1], dt[:, ::-1],
                        -1.0, Op.max, Op.max)
                    vmask(0, L, k, k)
                # a_k = (M[0] < 0)
                nc.vector.tensor_scalar(
                    acc[:, 2 * NSUM + k:2 * NSUM + k + 1], M[:, 0:1],
                    0.0, 1.0, Op.is_lt, Op.mult)

            # all labels upfront (bufs=4; zl_k gates each tile's DVE block),
            # ot chunks two tiles deep
            lts = [issue_lab(k, first=(_r == 0 and k == 0))
                   for k in range(TILES)]
            ots = [issue_ot(0), issue_ot(1)]
            for k in range(TILES):
                compute(k, lts[k], ots[k], chunked=(k == TILES - 1))
                if k + 2 < TILES:
                    ots.append(issue_ot(k + 2))

        nc.sync.dma_start(res_d[:], acc[:])

    nc.compile()
    return nc


def _pre_tile() -> np.ndarray:
    import ml_dtypes
    j = np.arange(L, dtype=np.float64)
    pre2 = (-3.6 / np.log2(j + 2.0) - C_CONST).astype(ml_dtypes.bfloat16)
    return np.ascontiguousarray(np.tile(pre2[None, :], (P, 1)))


def _get_nc(repeat: int = 1):
    key = repeat
    if key not in _CACHE:
        _CACHE[key] = _build_nc(repeat=repeat)
    return _CACHE[key]


def make_in_maps(output: np.ndarray, labels: np.ndarray):
    pre = _pre_tile()
    in_maps = []
    for c in range(N_CORES):
        sl = slice(c * ROWS_PER_CORE, (c + 1) * ROWS_PER_CORE)
        in_maps.append({
            "out": np.ascontiguousarray(output[sl]).reshape(ROWS_PER_CORE, L * 2),
            "lab": np.ascontiguousarray(labels[sl]),
            "pre": pre,
        })
    return in_maps


def combine_res(res: np.ndarray) -> float:
    """res: [P, RES_COLS] -> this core's total loss sum (fp64 on host).

    Layout: S1 partials [0:7) = tiles 0-2 then tile-3 chunks c0..c3;
    S2 partials [7:14) same order; a flags [14:18) per tile.
    """
    r = res.astype(np.float64)
    s1 = np.stack([r[:, 0], r[:, 1], r[:, 2], r[:, 3:NSUM].sum(axis=1)], axis=1)
    s2 = np.stack([r[:, NSUM], r[:, NSUM + 1], r[:, NSUM + 2],
                   r[:, NSUM + 3:2 * NSUM].sum(axis=1)], axis=1)
    a = r[:, 2 * NSUM:2 * NSUM + TILES]
    return float((s1 + a * (s2 - s1)).sum())


def kernel(output: np.ndarray, labels: np.ndarray) -> np.ndarray:
    from concourse.bass_utils import run_bass_kernel_spmd

    nc = _get_nc(repeat=1)
    in_maps = make_in_maps(output, labels)
    r = run_bass_kernel_spmd(nc, in_maps, core_ids=list(range(N_CORES)))
    total = 0.0
    for res in r.results:
        total += combine_res(res["res"])
    return np.float32(total / B)


if __name__ == "__main__":
    # quick standalone run (full inputs, random)
    rng = np.random.default_rng(0)
    out = rng.standard_normal((B, L, 2)).astype(np.float32)
    lab = rng.integers(0, 2, size=(B, L)).astype(np.int32)
    print("loss:", kernel(out, lab))
